# revision 54
# baseline (speedup 1.0000x reference)
# Trainium2 Bass kernel for nn_EquShiftQ2DF3P40 (group-equivariant CNN + dynamic filter).
#
# Sharding: batch 256 -> 32 samples/core on 8 cores. All weights replicated
# except the first es_fc layer (16384x1024), which is K-split across cores
# (each core contracts a 2048-feature slice for ALL 256 samples) followed by a
# bf16 ReduceScatter along the batch dim so each core ends with es1
# pre-activations for exactly its own 32 samples.
#
# Schedule notes (v2):
# - im2col planes for conv1/ihc1 are pre-shifted on the host (img9/ih9) so the
#   device loads them as big contiguous DMAs instead of 84-byte bursts.
# - conv1 runs as 4 stride-2 phase convs; 2x2 maxpool = max over phases done
#   by psum-reading tensor_reduce, so relu+bias touch only pooled pixels.
# - the rep body is software-pipelined: ihc1->ihc2 staggered, then c1 chunks 0-1,
#   then es1 (whose weight stream rides the gpsimd queue and hides under c1),
#   then c1 chunk k || c2 samples of chunk k-2. The ReduceScatter overlaps c2.
# - big mid/late weights (w2*, w3, w4, wes2, wdf, wifp2) are held back from the
#   HBM-saturated ramp: gpsimd-queue loads sit behind a full wait-queue of
#   collective semaphores, and wifp2 chunks are gated on es1/c2 outputs.
#
# Compute layout notes:
# - convs run channels-on-partitions; 3x3 taps become accumulating matmuls with
#   shifted access patterns; conv2/ihc2 stack two dy taps on the partition axis
#   (a shifted copy of the input lives at the upper partitions) to raise K.
# - conv1/ihc1 have Cin=1, so a 9-partition im2col (9 shifted DMA copies of the
#   padded image) makes K=9.
# - the in-hand FC (6400->512) runs as 100 pixel-wise accumulating matmuls with
#   the per-pixel activation block as the stationary operand.
# - the dynamic-filter tail stays batch-major (samples on partitions) and runs
#   on the vector engine as broadcast-mul + segmented reduce.
import numpy as np
import ml_dtypes

import concourse.bacc as bacc
import concourse.mybir as mybir
from concourse.bass_utils import run_bass_kernel_spmd
from concourse import tile
import bass_rust

f32 = mybir.dt.float32
bf16 = mybir.dt.bfloat16
AF = mybir.ActivationFunctionType
ALU = mybir.AluOpType
bf = ml_dtypes.bfloat16

NCORES = 8
BC = 32  # samples per core
KSLICE = 16384 // NCORES  # es1 contraction slice per core


# ---------------------------------------------------------------- host prep
def _rot(x, g):
    return np.rot90(x, k=g, axes=(-2, -1))


def _sym(k):
    return 0.5 * (k + _rot(k, 2))


def _expand_tq(kappa):
    kappa = _sym(kappa)
    Co, Ci, kh, kw = kappa.shape
    W = np.stack([_rot(kappa, g) for g in range(2)], axis=1)
    return W.reshape(Co * 2, Ci, kh, kw)


def _expand_qq(kappa):
    kappa = _sym(kappa)
    Co, Ci, F, kh, kw = kappa.shape
    W = np.stack([_rot(np.roll(kappa, g, axis=2), g) for g in range(F)], axis=1)
    return W.reshape(Co * F, Ci * F, kh, kw)


def _im2col9(x, stride, n):
    # (B,1,40,40) -> (9, B, n, n) pre-shifted im2col planes:
    # out[(dy,dx), s, y, x] = pad(x)[s, stride*y+dy, stride*x+dx]
    B = x.shape[0]
    p = np.zeros((B, 42, 42), np.float32)
    p[:, 1:41, 1:41] = x[:, 0]
    out = np.empty((9, B, n, n), np.float32)
    for dy in range(3):
        for dx in range(3):
            out[dy * 3 + dx] = p[:, dy:dy + stride * (n - 1) + 1:stride,
                                 dx:dx + stride * (n - 1) + 1:stride]
    return out


def host_prep(inputs):
    """Returns (shared_map, per_core_maps) of numpy arrays keyed by dram names."""
    obs = np.asarray(inputs["obs_encoding"], np.float32)
    patch = np.asarray(inputs["patch"], np.float32)
    B = obs.shape[0]

    W1e = _expand_tq(np.asarray(inputs["k1"], np.float32))        # (64,1,3,3)
    W2e = _expand_qq(np.asarray(inputs["k2"], np.float32))        # (128,64,3,3)
    W3e = _expand_qq(np.asarray(inputs["k3"], np.float32))        # (256,128,3,3)
    W4e = _expand_qq(np.asarray(inputs["k4"], np.float32))        # (64,256,3,3)
    W5e = _expand_qq(np.asarray(inputs["k5"], np.float32))        # (32,64,3,3)

    sh = {}
    # conv1 im2col weights (9,64): [(dy,dx), o]
    sh["w1"] = W1e[:, 0].reshape(64, 9).T.astype(bf).copy()
    sh["wi1"] = np.asarray(inputs["Wi1"], np.float32)[:, 0].reshape(32, 9).T.astype(bf).copy()
    # conv2 dy-stacked: w2a (3,128,128) rows=(dy0 ci | dy1 ci); w2b (3,64,128) dy2
    w2a = np.zeros((3, 128, 128), np.float32)
    w2b = np.zeros((3, 64, 128), np.float32)
    for dx in range(3):
        w2a[dx, 0:64] = W2e[:, :, 0, dx].T
        w2a[dx, 64:128] = W2e[:, :, 1, dx].T
        w2b[dx] = W2e[:, :, 2, dx].T
    sh["w2a"] = w2a.astype(bf)
    sh["w2ao"] = w2a[:, list(range(64, 128)) + list(range(64))].astype(bf).copy()
    sh["w2b"] = w2b.astype(bf)
    # conv3: (2,3,3,128,128) [mt][dy][dx][ci][o]
    w3 = np.zeros((2, 3, 3, 128, 128), np.float32)
    for mt in range(2):
        for dy in range(3):
            for dx in range(3):
                w3[mt, dy, dx] = W3e[mt * 128:(mt + 1) * 128, :, dy, dx].T
    sh["w3"] = w3.astype(bf)
    # conv4: (2,3,3,128,64) [kt][dy][dx][ci][o]
    w4 = np.zeros((2, 3, 3, 128, 64), np.float32)
    for kt in range(2):
        for dy in range(3):
            for dx in range(3):
                w4[kt, dy, dx] = W4e[:, kt * 128:(kt + 1) * 128, dy, dx].T
    sh["w4"] = w4.astype(bf)
    # conv5: (9,64,32)
    w5 = np.zeros((9, 64, 32), np.float32)
    for dy in range(3):
        for dx in range(3):
            w5[dy * 3 + dx] = W5e[:, :, dy, dx].T
    sh["w5"] = w5.astype(bf)
    # ihc2 dy-stacked (stride 2): wi2a (3,64,64) rows=(dy0 ci | dy1 ci); wi2b (3,32,64)
    Wi2 = np.asarray(inputs["Wi2"], np.float32)
    wi2a = np.zeros((3, 64, 64), np.float32)
    wi2b = np.zeros((3, 32, 64), np.float32)
    for dx in range(3):
        wi2a[dx, 0:32] = Wi2[:, :, 0, dx].T
        wi2a[dx, 32:64] = Wi2[:, :, 1, dx].T
        wi2b[dx] = Wi2[:, :, 2, dx].T
    sh["wi2a"] = wi2a.astype(bf)
    sh["wi2ao"] = wi2a[:, list(range(32, 64)) + list(range(32))].astype(bf).copy()
    sh["wi2b"] = wi2b.astype(bf)
    # in-hand FC pix-paired: (128,50,512): rows 0-63 = (ch, pix q), 64-127 = (ch, pix q+50)
    wif3 = np.asarray(inputs["Wif"], np.float32).reshape(64, 100, 512)
    sh["wifp2"] = np.concatenate([wif3[:, :50], wif3[:, 50:]], axis=0).astype(bf).copy()
    sh["wes2"] = np.asarray(inputs["Wes2"], np.float32).astype(bf)   # (1024,512)
    sh["wdf"] = np.asarray(inputs["Wdf"], np.float32).astype(bf)     # (1024,528)
    sh["ident"] = np.eye(32, dtype=np.float32).astype(bf)

    # biases / tail constants (f32)
    b1e = np.repeat(np.asarray(inputs["b1"], np.float32), 2)
    b2e = np.repeat(np.asarray(inputs["b2"], np.float32), 2)
    b3e = np.repeat(np.asarray(inputs["b3"], np.float32), 2)
    b4e = np.repeat(np.asarray(inputs["b4"], np.float32), 2)
    b5e = np.repeat(np.asarray(inputs["b5"], np.float32), 2)
    sh["bc1"] = np.concatenate([b1e, b1e]).reshape(128, 1).copy()
    sh["bc2"] = b2e.reshape(128, 1).copy()
    sh["bc3"] = b3e.reshape(128, 2, order="F").copy()  # [p, mt] with b3e[mt*128+p]
    sh["bc4"] = b4e.reshape(64, 1).copy()
    sh["b5rep"] = np.tile(b5e, (BC, 1)).copy()                       # (32,32)
    sh["bi1c"] = np.tile(np.asarray(inputs["bi1"], np.float32), 4).reshape(128, 1).copy()
    sh["bi2co"] = np.tile(np.asarray(inputs["bi2"], np.float32), 2).reshape(128, 1).copy()
    sh["bi2c"] = np.tile(np.asarray(inputs["bi2"], np.float32), 2).reshape(128, 1).copy()
    sh["bes1t"] = np.asarray(inputs["bes1"], np.float32).reshape(8, 128).T.copy()   # (128,8)
    sh["bes2t"] = np.asarray(inputs["bes2"], np.float32).reshape(4, 128).T.copy()   # (128,4)
    sh["bifrep"] = np.tile(np.asarray(inputs["bif"], np.float32), (BC, 1)).copy()   # (32,512)
    sh["bdfrep"] = np.tile(np.asarray(inputs["bdf"], np.float32), (BC, 1)).copy()   # (32,528)
    kappa2 = np.asarray(inputs["kappa2"], np.float32)
    W2f = np.stack([np.roll(kappa2, g, axis=2) for g in range(2)], axis=1).reshape(4, 32)
    sh["w2rep"] = np.tile(W2f, (BC, 1, 1)).copy()                    # (32,4,32)
    sh["b2frep"] = np.tile(np.repeat(np.asarray(inputs["b2f"], np.float32), 2), (BC, 1)).copy()  # (32,4)

    # per-core tensors
    obsT = np.ascontiguousarray(obs.reshape(B, 16384).T)  # (16384, 256)
    wes1 = np.asarray(inputs["Wes1"], np.float32)          # (16384, 1024)
    img9 = _im2col9(patch[:, :1], 1, 40).astype(bf)        # (9, B, 40, 40)
    ih9 = _im2col9(patch[:, 1:], 2, 20).astype(bf)         # (9, B, 20, 20)
    per_core = []
    for c in range(NCORES):
        m = dict(sh)
        m["obsT"] = obsT[c * KSLICE:(c + 1) * KSLICE].astype(bf)
        m["wes1"] = wes1[c * KSLICE:(c + 1) * KSLICE].astype(bf)
        sl = slice(c * BC, (c + 1) * BC)
        m["img9"] = np.ascontiguousarray(img9[:, sl])
        m["ih9"] = np.ascontiguousarray(ih9[:, sl])
        per_core.append(m)
    return per_core


# ---------------------------------------------------------------- bass build
def build(debug=(), reps=1, sim=False):
    nc = bacc.Bacc("TRN2", target_bir_lowering=False, debug=False, num_devices=NCORES)

    D = {}

    def din(name, shape, dt=bf16):
        D[name] = nc.dram_tensor(name, list(shape), dt, kind="ExternalInput")
        return D[name]

    obsT_d = din("obsT", (KSLICE, 256))
    wes1_d = din("wes1", (KSLICE, 1024))
    img9_d = din("img9", (9, BC, 40, 40))
    ih9_d = din("ih9", (9, BC, 20, 20))
    w1_d = din("w1", (9, 64))
    wi1_d = din("wi1", (9, 32))
    w2a_d = din("w2a", (3, 128, 128))
    w2ao_d = din("w2ao", (3, 128, 128))
    w2b_d = din("w2b", (3, 64, 128))
    w3_d = din("w3", (2, 3, 3, 128, 128))
    w4_d = din("w4", (2, 3, 3, 128, 64))
    w5_d = din("w5", (9, 64, 32))
    wi2a_d = din("wi2a", (3, 64, 64))
    wi2ao_d = din("wi2ao", (3, 64, 64))
    wi2b_d = din("wi2b", (3, 32, 64))
    wifp2_d = din("wifp2", (128, 50, 512))
    wes2_d = din("wes2", (1024, 512))
    wdf_d = din("wdf", (1024, 528))
    ident_d = din("ident", (32, 32))
    for nm, shp in [("bc1", (128, 1)), ("bc2", (128, 1)), ("bc3", (128, 2)),
                    ("bc4", (64, 1)), ("b5rep", (BC, 32)), ("bi1c", (128, 1)),
                    ("bi2c", (128, 1)), ("bes1t", (128, 8)), ("bes2t", (128, 4)),
                    ("bifrep", (BC, 512)), ("bdfrep", (BC, 528)),
                    ("w2rep", (BC, 4, 32)), ("b2frep", (BC, 4))]:
        din(nm, shp, f32)

    cc_ins = [nc.dram_tensor(f"cc_in{r}", [256, 1024], bf16) for r in range(reps)]
    cc_outs = [nc.dram_tensor(f"cc_out{r}", [BC, 1024], bf16) for r in range(reps)]
    out_d = nc.dram_tensor("out", [BC, 4], f32, kind="ExternalOutput")

    dbg_handles = {}

    def dbg(name, shape, dt):
        dbg_handles[name] = nc.dram_tensor(name, list(shape), dt, kind="ExternalOutput")
        return dbg_handles[name]

    with tile.TileContext(nc) as tc:
        with tc.tile_pool(name="pw", bufs=1) as pw, \
             tc.tile_pool(name="pwif", bufs=2) as pwif, \
             tc.tile_pool(name="psum", bufs=2, space="PSUM") as psp:

            _sc = [None]

            def mark(name):
                if _sc[0] is not None:
                    nc.leave_named_scope(_sc[0][0], _sc[0][1], False)
                    _sc[0] = None
                if name:
                    sid, _ = nc.enter_named_scope(name, False)
                    _sc[0] = (name, sid)

            # ---------- persistent weight tiles
            mark("wload")
            def ld(name, shape, src_ap, dt=bf16, pool=None, eng=None):
                t = (pool or pw).tile(list(shape), dt, tag=name)
                (eng or nc.sync).dma_start(out=t[:], in_=src_ap)
                return t

            w1_t = ld("w1", (9, 64), w1_d[:])
            wi1_t = ld("wi1", (9, 32), wi1_d[:])
            bias_t = {}
            for nm, shp in [("bc1", (128, 1)), ("bi1c", (128, 1))]:
                bias_t[nm] = ld(nm, shp, D[nm][:], dt=f32)

            def load_early_weights():
                # ihc2 weights on the scalar queue: a few dispatches ahead
                # of the first ihc1 eviction, done well before first use.
                se = nc.scalar
                global_w = {}
                global_w["wi2a"] = ld("wi2a", (64, 3, 64), wi2a_d[:].rearrange("d p o -> p d o"), eng=se)
                global_w["wi2ao"] = ld("wi2ao", (64, 3, 64), wi2ao_d[:].rearrange("d p o -> p d o"), eng=se)
                wi2b = pw.tile([64, 3, 64], bf16, tag="wi2b")
                se.dma_start(out=wi2b[32:64, :, :], in_=wi2b_d[:].rearrange("d p o -> p d o"))
                se.dma_start(out=wi2b[0:32, :, :], in_=wi2b_d[:].rearrange("d p o -> p d o"))
                global_w["wi2b"] = wi2b
                for nm, shp in [("bi2c", (128, 1))]:
                    bias_t[nm] = ld(nm, shp, D[nm][:], dt=f32, eng=se)
                return global_w

            def load_late_weights():
                # mid/tail-phase weights on the gpsimd queue, emitted behind a
                # full wait-queue of collective semaphores so their transfers
                # don't contend with es1/im2col streams during the ramp.
                ge = nc.gpsimd
                global_w = {}
                global_w["w2a"] = ld("w2a", (128, 3, 128), w2a_d[:].rearrange("d p o -> p d o"), eng=ge)
                global_w["w2ao"] = ld("w2ao", (128, 3, 128), w2ao_d[:].rearrange("d p o -> p d o"), eng=ge)
                w2b = pw.tile([128, 3, 128], bf16, tag="w2b")
                ge.dma_start(out=w2b[64:128, :, :], in_=w2b_d[:].rearrange("d p o -> p d o"))
                ge.dma_start(out=w2b[0:64, :, :], in_=w2b_d[:].rearrange("d p o -> p d o"))
                global_w["w2b"] = w2b
                global_w["w3"] = ld("w3", (128, 18, 128), w3_d[:].rearrange("m y x p o -> p (m y x) o"), eng=ge)
                global_w["w4"] = ld("w4", (128, 18, 64), w4_d[:].rearrange("k y x p o -> p (k y x) o"), eng=ge)
                global_w["w5"] = ld("w5", (64, 9, 32), w5_d[:].rearrange("q p o -> p q o"), eng=ge)
                global_w["wes2"] = ld("wes2", (128, 8, 512), wes2_d[:].rearrange("(t p) o -> p t o", p=128), eng=ge)
                global_w["wdf"] = ld("wdf", (128, 8, 528), wdf_d[:].rearrange("(t p) o -> p t o", p=128), eng=ge)
                global_w["ident"] = ld("ident", (32, 32), ident_d[:], eng=ge)
                for nm, shp in [("bc2", (128, 1)), ("bc3", (128, 2)),
                                ("bc4", (64, 1)), ("b5rep", (BC, 32)),
                                ("bes1t", (128, 8)), ("bes2t", (128, 4)),
                                ("bifrep", (BC, 512)), ("bdfrep", (BC, 528)),
                                ("w2rep", (BC, 4, 32)), ("b2frep", (BC, 4))]:
                    bias_t[nm] = ld(nm, shp, D[nm][:], dt=f32, eng=ge)
                return global_w

            for rep in range(reps):
                # ================= conv stage pools ============================
                with tc.tile_pool(name="pconv", bufs=1) as pc:
                    # parity-split padded inputs: E tiles keep base rows on the
                    # lower partition half (shifted copy above); O tiles are
                    # mirrored so lane-locked evictions stay partition-aligned.
                    xihE = pc.tile([64, 16, 22, 22], bf16, tag="xihE")
                    xihO = pc.tile([64, 16, 22, 22], bf16, tag="xihO")
                    x1pE = pc.tile([128, 16, 22, 22], bf16, tag="x1pE")
                    x1pO = pc.tile([128, 16, 22, 22], bf16, tag="x1pO")
                    hst = pc.tile([128, 16, 10, 10], bf16, tag="hst")
                    h_lin2 = pc.tile([128, BC, 50], bf16, tag="h_lin2")
                    x2 = pc.tile([128, BC, 10, 10], bf16, tag="x2")
                    x3 = pc.tile([128, 2, BC, 8, 8], bf16, tag="x3")
                    x4 = pc.tile([64, BC, 3, 3], bf16, tag="x4")

                    # border zeroing (interiors written by conv evictions)
                    for t_, p0, p1_ in ((xihE, 0, 32), (xihO, 32, 64), (x1pE, 0, 64), (x1pO, 64, 128)):
                        nc.vector.memset(t_[p0:p1_, :, 0:1, :], 0.0)
                        nc.vector.memset(t_[p0:p1_, :, 21:22, :], 0.0)
                        nc.vector.memset(t_[p0:p1_, :, :, 0:1], 0.0)
                        nc.vector.memset(t_[p0:p1_, :, :, 21:22], 0.0)

                    gw = load_early_weights()
                    wi2a_t, wi2ao_t, wi2b_t = gw["wi2a"], gw["wi2ao"], gw["wi2b"]

                    with tc.tile_pool(name="pim", bufs=2) as pim, \
                         tc.tile_pool(name="pev", bufs=2) as pev, \
                         tc.tile_pool(name="pcv", bufs=2) as pcv, \
                         tc.tile_pool(name="pes", bufs=1) as pes, \
                         tc.tile_pool(name="pes2", bufs=2) as pes2, \
                         tc.tile_pool(name="pes2b", bufs=2) as pes2b:
                        psc2 = None  # c2's dedicated psum pool, opened after es1

                        def ihc1_chunk(g):
                            t9i = pim.tile([9, 4, 20, 20], bf16, tag="t9i")
                            nc.sync.dma_start(out=t9i[:], in_=ih9_d[:, g * 4:(g + 1) * 4])
                            for mm in range(2):
                                m = g * 2 + mm
                                pp = psp.tile([64, 20, 20], f32, tag="mm")
                                for j in range(2):
                                    nc.tensor.matmul(pp[32 * j:32 * (j + 1), :, :], wi1_t[:],
                                                     t9i[:, 2 * mm + j, :, :],
                                                     start=True, stop=True, tile_position=(0, 32 * j))
                                nc.scalar.activation(xihE[0:32, m, 1:21, 1:21], pp[0:32, :, :],
                                                     AF.Relu, bias=bias_t["bi1c"][0:32, 0:1])
                                nc.vector.tensor_scalar(xihO[32:64, m, 1:21, 1:21], pp[32:64, :, :],
                                                        bias_t["bi1c"][32:64, 0:1], 0.0, ALU.add, ALU.max)
                            q0 = g * 2
                            nc.sync.dma_start(out=xihE[32:64, q0:q0 + 2, 0:21, :],
                                              in_=xihE[0:32, q0:q0 + 2, 1:22, :])
                            nc.sync.dma_start(out=xihO[0:32, q0:q0 + 2, 0:21, :],
                                              in_=xihO[32:64, q0:q0 + 2, 1:22, :])

                        def ihc2_group(g):
                            p0 = g * 4
                            pp = psp.tile([128, 4, 10, 10], f32, tag="mm")
                            # even members (xihE), odd members (xihO)
                            for dx in range(3):
                                nc.tensor.matmul(pp[0:64, :, :, :], wi2a_t[:, dx, :],
                                                 xihE[0:64, p0:p0 + 4, 0:20:2, dx:dx + 20:2],
                                                 start=(dx == 0), stop=False, tile_position=(0, 0))
                            for dx in range(3):
                                nc.tensor.matmul(pp[0:64, :, :, :], wi2b_t[32:64, dx, :],
                                                 xihE[32:64, p0:p0 + 4, 1:21:2, dx:dx + 20:2],
                                                 start=False, stop=(dx == 2), tile_position=(32, 0))
                            for dx in range(3):
                                nc.tensor.matmul(pp[64:128, :, :, :], wi2ao_t[:, dx, :],
                                                 xihO[0:64, p0:p0 + 4, 0:20:2, dx:dx + 20:2],
                                                 start=(dx == 0), stop=False, tile_position=(0, 64))
                            for dx in range(3):
                                nc.tensor.matmul(pp[64:128, :, :, :], wi2b_t[0:32, dx, :],
                                                 xihO[0:32, p0:p0 + 4, 1:21:2, dx:dx + 20:2],
                                                 start=False, stop=(dx == 2), tile_position=(0, 64))
                            nc.scalar.activation(hst[:, p0:p0 + 4, :, :], pp[:],
                                                 AF.Relu, bias=bias_t["bi2c"][:, 0:1])

                        def c1_chunk(k):
                            # pool-before-activation: conv1 as 4 stride-2 phase
                            # convs; 2x2 maxpool = max over phases (exact: bias
                            # add and relu commute with max). Act work halves.
                            t9 = pim.tile([9, 4, 40, 40], bf16, tag="t9")
                            nc.sync.dma_start(out=t9[:], in_=img9_d[:, k * 4:(k + 1) * 4])
                            for pr in range(2):
                                m = k * 2 + pr
                                ppab = []
                                for a in range(2):
                                    pp = psp.tile([128, 2, 512], f32, tag="mm")
                                    for b in range(2):
                                        for j in range(2):
                                            nc.tensor.matmul(
                                                pp[64 * j:64 * (j + 1), b, 0:400]
                                                .rearrange("p (y x) -> p y x", x=20),
                                                w1_t[:],
                                                t9[:, pr * 2 + j, a:40:2, b:40:2],
                                                start=True, stop=True, tile_position=(0, 64 * j))
                                    ppab.append(pp)
                                e0 = pev.tile([128, 400], bf16, tag="c1e0")
                                nc.vector.tensor_reduce(
                                    e0[:], ppab[0][:, :, 0:400].rearrange("p b x -> p x b"),
                                    mybir.AxisListType.X, ALU.max)
                                e1 = pev.tile([128, 400], bf16, tag="c1e1")
                                nc.vector.tensor_reduce(
                                    e1[:], ppab[1][:, :, 0:400].rearrange("p b x -> p x b"),
                                    mybir.AxisListType.X, ALU.max)
                                fin = pev.tile([128, 400], bf16, tag="c1f")
                                nc.vector.tensor_tensor(fin[:], e0[:], e1[:], ALU.max)
                                finv = fin[:].rearrange("p (y x) -> p y x", x=20)
                                nc.scalar.activation(x1pE[0:64, m, 1:21, 1:21], finv[0:64],
                                                     AF.Relu, bias=bias_t["bc1"][0:64, 0:1])
                                nc.scalar.activation(x1pO[64:128, m, 1:21, 1:21], finv[64:128],
                                                     AF.Relu, bias=bias_t["bc1"][64:128, 0:1])
                            m0 = k * 2
                            nc.sync.dma_start(out=x1pE[64:128, m0:m0 + 2, 0:21, :],
                                              in_=x1pE[0:64, m0:m0 + 2, 1:22, :])
                            nc.sync.dma_start(out=x1pO[0:64, m0:m0 + 2, 0:21, :],
                                              in_=x1pO[64:128, m0:m0 + 2, 1:22, :])

                        def c2_sample(s):
                            m = s // 2
                            if s % 2 == 0:
                                X, wa, wbl = x1pE, w2a_t, w2b_t[64:128, :, :]
                                dy2 = lambda dx: X[64:128, m, 1:21, dx:dx + 20]
                            else:
                                X, wa, wbl = x1pO, w2ao_t, w2b_t[0:64, :, :]
                                dy2 = lambda dx: X[0:64, m, 1:21, dx:dx + 20]
                            pp = psc2.tile([128, 20, 20], f32, tag="c2m")
                            for dx in range(3):
                                nc.tensor.matmul(pp[:], wa[:, dx, :], X[:, m, 0:20, dx:dx + 20],
                                                 start=(dx == 0), stop=False)
                            for dx in range(3):
                                nc.tensor.matmul(pp[:], wbl[:, dx], dy2(dx),
                                                 start=False, stop=(dx == 2))
                            t2 = pcv.tile([128, 20, 20], bf16, tag="c2e")
                            nc.scalar.activation(t2[:], pp[:], AF.Relu, bias=bias_t["bc2"][:, 0:1])
                            h2 = pcv.tile([128, 20, 10], bf16, tag="c2h")
                            nc.vector.tensor_tensor(h2[:], t2[:, :, 0:20:2], t2[:, :, 1:20:2], ALU.max)
                            nc.vector.tensor_tensor(x2[:, s, :, :], h2[:, 0:20:2, :], h2[:, 1:20:2, :], ALU.max)

                        # ---------- ihc1 -> ihc2 software pipeline
                        mark("ihc1")
                        for g in range(8):
                            ihc1_chunk(g)
                            if g % 2 == 1:
                                ihc2_group((g - 1) // 2)
                        # h_lin2[(pixgroup, ch), s, q]: rows 0-63 = pix q, 64-127 = pix q+50
                        nc.scalar.dma_start(out=h_lin2[0:64, 0:32:2, :],
                                            in_=hst[0:64, :, 0:5, :].rearrange("p k a b -> p k (a b)"))
                        nc.scalar.dma_start(out=h_lin2[0:64, 1:32:2, :],
                                            in_=hst[64:128, :, 0:5, :].rearrange("p k a b -> p k (a b)"))
                        nc.scalar.dma_start(out=h_lin2[64:128, 0:32:2, :],
                                            in_=hst[0:64, :, 5:10, :].rearrange("p k a b -> p k (a b)"))
                        nc.scalar.dma_start(out=h_lin2[64:128, 1:32:2, :],
                                            in_=hst[64:128, :, 5:10, :].rearrange("p k a b -> p k (a b)"))
                        if "dbg_hlin" in debug:
                            nc.sync.dma_start(out=dbg("dbg_hlin", (128, BC, 50), bf16)[:], in_=h_lin2[:])

                        mark("c1")
                        c1_chunk(0)
                        c1_chunk(1)

                        # ---------- es1: K-split streaming matmul, emitted after
                        # two c1 chunks so its PE stream overlaps c1 evictions;
                        # obsT/wes1 chunks ride the gpsimd queue, which may block
                        # on pool-rotation semaphores without stalling others.
                        mark("es1")
                        pesp_cm = tc.tile_pool(name="pesp", bufs=1, space="PSUM")
                        pesp = pesp_cm.__enter__()
                        acc = pesp.tile([128, 2, 2, 512], f32, tag="es1p")
                        wc, obc = None, None
                        for kt in range(16):
                            if kt % 2 == 0:
                                obc = pes2b.tile([128, 2, 256], bf16, tag="obc")
                                nc.gpsimd.dma_start(
                                    out=obc[:],
                                    in_=obsT_d[128 * kt:128 * (kt + 2), :]
                                    .rearrange("(t p) b -> p t b", p=128))
                                wc = pes2.tile([128, 2, 1024], bf16, tag="wes1c")
                                nc.gpsimd.dma_start(
                                    out=wc[:],
                                    in_=wes1_d[128 * kt:128 * (kt + 2), :]
                                    .rearrange("(t p) o -> p t o", p=128))
                            for bb in range(2):
                                for nt in range(2):
                                    nc.tensor.matmul(acc[:, bb, nt, :],
                                                     obc[:, kt % 2, bb * 128:(bb + 1) * 128],
                                                     wc[:, kt % 2, nt * 512:(nt + 1) * 512],
                                                     start=(kt == 0), stop=(kt == 15))
                        es1s = pes.tile([128, 2, 2, 512], bf16, tag="es1s")
                        nc.vector.tensor_copy(es1s[:], acc[:])
                        pesp_cm.__exit__(None, None, None)
                        # dedicated 3-deep single-bank psum rotation for c2 so
                        # its matmuls never wait on c1's eviction slots
                        psc2_cm = tc.tile_pool(name="psc2", bufs=3, space="PSUM")
                        psc2 = psc2_cm.__enter__()
                        nc.gpsimd.dma_start(out=cc_ins[rep][:].rearrange("(bb p) (nt o) -> p bb nt o", p=128, o=512),
                                            in_=es1s[:])
                        # first wifp2 chunk, gated behind es1's eviction so the
                        # 6.5MB stream doesn't contend with the front loads
                        wifc_pre = []
                        wg = pwif.tile([128, 10, 512], bf16, tag="wifc")
                        nc.vector.tensor_copy(wg[0:1, 0, 0:2], es1s[0:1, 0, 0, 0:2])
                        nc.sync.dma_start(out=wg[:], in_=wifp2_d[:, 0:10, :])
                        wifc_pre.append(wg)
                        if sim:
                            nc.gpsimd.dma_start(out=cc_outs[rep][:], in_=cc_ins[rep][0:BC, :])
                        else:
                            nc.gpsimd.collective_compute(
                                "ReduceScatter", ALU.add, replica_groups=[list(range(NCORES))],
                                ins=[cc_ins[rep][:]], outs=[cc_outs[rep][:]])
                        esb = pw.tile([BC, 1024], bf16, tag="esb")
                        nc.gpsimd.dma_start(out=esb[:], in_=cc_outs[rep][:])
                        # 4th parked waiter fills the gpsimd wait queue so the
                        # late-weight DMAs below cannot bypass the collective
                        # and steal HBM bandwidth during the ramp.
                        scrg = pw.tile([1, 4], bf16, tag="scrg")
                        nc.gpsimd.dma_start(out=scrg[:], in_=esb[0:1, 0:4])
                        gw_late = load_late_weights()
                        gw.update(gw_late)
                        w2a_t, w2ao_t, w2b_t = gw["w2a"], gw["w2ao"], gw["w2b"]
                        if "dbg_es1" in debug:
                            nc.sync.dma_start(out=dbg("dbg_es1", (BC, 1024), bf16)[:], in_=esb[:])

                        # ---------- c1 -> c2 software pipeline: c2's PE-dense
                        # blocks fill the PE while c1's evictions run on Act/DVE
                        mark("c2")
                        for k in range(2, 11):
                            if k < 8:
                                c1_chunk(k)
                            if k >= 3:
                                for s in range(4 * (k - 3), 4 * (k - 2)):
                                    c2_sample(s)
                            if k == 3:
                                # second wifp2 chunk released once c2 is underway
                                wg2 = pwif.tile([128, 10, 512], bf16, tag="wifc")
                                nc.vector.tensor_copy(wg2[0:1, 0, 0:2], x2[0:1, 0, 0, 0:2])
                                nc.sync.dma_start(out=wg2[:], in_=wifp2_d[:, 10:20, :])
                                wifc_pre.append(wg2)
                        if "dbg_x2" in debug:
                            nc.sync.dma_start(out=dbg("dbg_x2", (128, BC, 10, 10), bf16)[:], in_=x2[:])
                        psc2_cm.__exit__(None, None, None)

                    w3_t, w4_t, w5_t = gw["w3"], gw["w4"], gw["w5"]
                    wes2_t, wdf_t, ident_t = gw["wes2"], gw["wdf"], gw["ident"]

                    with tc.tile_pool(name="psacc", bufs=1, space="PSUM") as psacc, \
                         tc.tile_pool(name="pssm", bufs=2, space="PSUM") as pssm:
                        # ---------- ihv: 50 pix-paired accumulating matmuls (K=128)
                        mark("ihv")
                        p_ihv = psacc.tile([BC, 512], f32, tag="ihv")
                        for ci, q0 in enumerate(range(0, 50, 10)):
                            if ci < 2:
                                wifc = wifc_pre[ci]
                            else:
                                wifc = pwif.tile([128, 10, 512], bf16, tag="wifc")
                                nc.sync.dma_start(out=wifc[:], in_=wifp2_d[:, q0:q0 + 10, :])
                            for q in range(10):
                                qg = q0 + q
                                nc.tensor.matmul(p_ihv[:], h_lin2[:, :, qg], wifc[:, q, :],
                                                 start=(qg == 0), stop=(qg == 49))
                        ihv_f = pc.tile([BC, 512], f32, tag="ihv_f")
                        nc.vector.tensor_tensor(ihv_f[:], p_ihv[:], bias_t["bifrep"][:], ALU.add)
                        ihvb = pc.tile([BC, 512], bf16, tag="ihvb")
                        nc.vector.tensor_scalar(ihvb[:], ihv_f[:], 0.0, None, ALU.max)
                        if "dbg_ihv" in debug:
                            nc.sync.dma_start(out=dbg("dbg_ihv", (BC, 512), bf16)[:], in_=ihvb[:])

                        catT = pw.tile([128, 8, BC], bf16, tag="catT")
                        for k in range(4):
                            pt = pssm.tile([128, BC], bf16, tag="sm")
                            nc.tensor.transpose(pt[:], ihvb[:, 128 * k:128 * (k + 1)], ident_t[:])
                            nc.vector.tensor_copy(catT[:, 4 + k, :], pt[:])

                        # ---------- es tail (hoisted): transpose RS output, bias+relu, es2
                        mark("estail")
                        esT = pw.tile([128, 8, BC], bf16, tag="esT")
                        for t in range(8):
                            pt = pssm.tile([128, BC], bf16, tag="sm")
                            nc.tensor.transpose(pt[:], esb[:, 128 * t:128 * (t + 1)], ident_t[:])
                            nc.vector.tensor_scalar(esT[:, t, :], pt[:], bias_t["bes1t"][:, t:t + 1],
                                                    0.0, ALU.add, ALU.max)
                        if "dbg_esT" in debug:
                            nc.sync.dma_start(out=dbg("dbg_esT", (128, 8, BC), bf16)[:], in_=esT[:])
                        for mt in range(4):
                            pp = pssm.tile([128, BC], f32, tag="sm")
                            for t in range(8):
                                nc.tensor.matmul(pp[:], wes2_t[:, t, mt * 128:(mt + 1) * 128], esT[:, t, :],
                                                 start=(t == 0), stop=(t == 7))
                            nc.vector.tensor_scalar(catT[:, mt, :], pp[:], bias_t["bes2t"][:, mt:mt + 1],
                                                    0.0, ALU.add, ALU.max)
                        if "dbg_catT" in debug:
                            nc.sync.dma_start(out=dbg("dbg_catT", (128, 8, BC), bf16)[:], in_=catT[:])

                        # ---------- df: dynamic filter weights (batch-major psum)
                        mark("df")
                        pdf1 = pssm.tile([BC, 512], f32, tag="sm")
                        pdf2 = psacc.tile([BC, 16], f32, tag="ihv")
                        for t in range(8):
                            nc.tensor.matmul(pdf1[:], catT[:, t, :], wdf_t[:, t, 0:512],
                                             start=(t == 0), stop=(t == 7))
                        for t in range(8):
                            nc.tensor.matmul(pdf2[:], catT[:, t, :], wdf_t[:, t, 512:528],
                                             start=(t == 0), stop=(t == 7))
                        wb_sb = pc.tile([BC, 528], f32, tag="wb_sb")
                        nc.vector.tensor_tensor(wb_sb[:, 0:512], pdf1[:], bias_t["bdfrep"][:, 0:512], ALU.add)
                        nc.vector.tensor_tensor(wb_sb[:, 512:528], pdf2[:], bias_t["bdfrep"][:, 512:528], ALU.add)
                        if "dbg_wb" in debug:
                            nc.sync.dma_start(out=dbg("dbg_wb", (BC, 528), f32)[:], in_=wb_sb[:])

                        # ---------- c3
                        mark("c3")
                        for mt in range(2):
                            for sg in range(4):
                                pp = psp.tile([128, 8, 8, 8], f32, tag="mm")
                                first = True
                                for dy in range(3):
                                    for dx in range(3):
                                        nc.tensor.matmul(pp[:], w3_t[:, mt * 9 + dy * 3 + dx, :],
                                                         x2[:, sg * 8:(sg + 1) * 8, dy:dy + 8, dx:dx + 8],
                                                         start=first, stop=(dy == 2 and dx == 2))
                                        first = False
                                nc.scalar.activation(x3[:, mt, sg * 8:(sg + 1) * 8, :, :], pp[:],
                                                     AF.Relu, bias=bias_t["bc3"][:, mt:mt + 1])
                        if "dbg_x3" in debug:
                            nc.sync.dma_start(out=dbg("dbg_x3", (128, 2, BC, 8, 8), bf16)[:], in_=x3[:])

                        # ---------- c4 + pool
                        mark("c4")
                        for sg in range(4):
                            pp = psp.tile([64, 8, 6, 6], f32, tag="mm")
                            first = True
                            for kt in range(2):
                                for dy in range(3):
                                    for dx in range(3):
                                        nc.tensor.matmul(pp[:], w4_t[:, kt * 9 + dy * 3 + dx, :],
                                                         x3[:, kt, sg * 8:(sg + 1) * 8, dy:dy + 6, dx:dx + 6],
                                                         start=first, stop=(kt == 1 and dy == 2 and dx == 2))
                                        first = False
                            t4 = pc.tile([64, 8, 6, 6], bf16, tag="c4e")
                            nc.scalar.activation(t4[:], pp[:], AF.Relu, bias=bias_t["bc4"][:, 0:1])
                            h4 = pc.tile([64, 8, 6, 3], bf16, tag="c4h")
                            nc.vector.tensor_tensor(h4[:], t4[:, :, :, 0:6:2], t4[:, :, :, 1:6:2], ALU.max)
                            nc.vector.tensor_tensor(x4[:, sg * 8:(sg + 1) * 8, :, :],
                                                    h4[:, :, 0:6:2, :], h4[:, :, 1:6:2, :], ALU.max)

                        # ---------- c5 (batch-major out: samples on partitions)
                        mark("c5")
                        pp5 = pssm.tile([BC, 32], f32, tag="sm")
                        for q in range(9):
                            dy, dx = divmod(q, 3)
                            nc.tensor.matmul(pp5[:], x4[:, :, dy, dx], w5_t[:, q, :],
                                             start=(q == 0), stop=(q == 8))
                        xs_t = pc.tile([BC, 16, 2], f32, tag="xs")
                        xs_p = pc.tile([BC, 16, 2], f32, tag="xs_p")
                        nc.vector.tensor_tensor(xs_p[:], pp5[:].rearrange("p (a b) -> p a b", b=2),
                                                bias_t["b5rep"][:].rearrange("p (a b) -> p a b", b=2), ALU.add)
                        nc.vector.tensor_scalar(xs_t[:], xs_p[:], 0.0, None, ALU.max)
                        xg1 = pc.tile([BC, 16, 2], f32, tag="xg1")
                        nc.vector.tensor_copy(xg1[:], xs_t[:, :, ::-1])
                        if "dbg_xf" in debug:
                            nc.sync.dma_start(out=dbg("dbg_xf", (BC, 16, 2), f32)[:], in_=xs_t[:])

                        # ---------- dynamic 1x1 group conv tail (all DVE)
                        mark("tail")
                        wbv = wb_sb[:, 0:512].rearrange("p (o j) -> p o j", j=32)
                        tmp0 = pc.tile([BC, 16, 32], f32, tag="tmp0")
                        tmp1 = pc.tile([BC, 16, 32], f32, tag="tmp1")
                        xb0 = xs_t[:].rearrange("p a b -> p (a b)").unsqueeze(1).broadcast_to((BC, 16, 32))
                        xb1 = xg1[:].rearrange("p a b -> p (a b)").unsqueeze(1).broadcast_to((BC, 16, 32))
                        nc.vector.tensor_mul(tmp0[:], wbv, xb0)
                        nc.vector.tensor_mul(tmp1[:], wbv, xb1)
                        featr = pc.tile([BC, 16, 2], f32, tag="featr")
                        f0 = pc.tile([BC, 16], f32, tag="f0")
                        f1 = pc.tile([BC, 16], f32, tag="f1")
                        nc.vector.tensor_reduce(f0[:], tmp0[:], mybir.AxisListType.X, ALU.add)
                        nc.vector.tensor_reduce(f1[:], tmp1[:], mybir.AxisListType.X, ALU.add)
                        nc.vector.tensor_tensor(featr[:, :, 0], f0[:], wb_sb[:, 512:528], ALU.add)
                        nc.vector.tensor_tensor(featr[:, :, 1], f1[:], wb_sb[:, 512:528], ALU.add)
                        nc.vector.tensor_scalar(featr[:], featr[:], 0.0, None, ALU.max)
                        fb_ = featr[:].rearrange("p a b -> p (a b)").unsqueeze(1).broadcast_to((BC, 4, 32))
                        tmp2 = pc.tile([BC, 4, 32], f32, tag="tmp2")
                        nc.vector.tensor_mul(tmp2[:], bias_t["w2rep"][:], fb_)
                        o4_t = pc.tile([BC, 4], f32, tag="o4")
                        nc.vector.tensor_reduce(o4_t[:], tmp2[:], mybir.AxisListType.X, ALU.add)
                        outsb = pc.tile([BC, 4], f32, tag="outsb")
                        nc.vector.tensor_tensor(outsb[:], o4_t[:], bias_t["b2frep"][:], ALU.add)
                        nc.sync.dma_start(out=out_d[:], in_=outsb[:])
                        mark(None)

    nc.compile()
    return nc, dbg_handles


# ---------------------------------------------------------------- run
_CACHE = {}


def _get_module(debug=(), reps=1, sim=False):
    key = (tuple(sorted(debug)), reps, sim)
    if key not in _CACHE:
        _CACHE[key] = build(debug, reps, sim)
    return _CACHE[key]


def run(inputs, debug=()):
    nc, dbg_handles = _get_module(debug)
    in_maps = host_prep(inputs)
    res = run_bass_kernel_spmd(nc, in_maps, list(range(NCORES)))
    return res


def kernel(**inputs):
    res = run(inputs)
    out = np.concatenate([np.asarray(res.results[c]["out"], np.float32) for c in range(NCORES)], axis=0)
    return out.reshape(256, 2, 2)


# ---------------------------------------------------------------- timing
def make_runner(nc, in_maps):
    """Builds a reusable jitted executor for `nc` (mirrors
    bass2jax.run_bass_via_pjrt's multi-core path) with device-resident inputs.
    Returns a zero-arg callable that executes once and blocks."""
    import jax
    import numpy as _np
    from jax.sharding import Mesh, PartitionSpec
    from jax.experimental.shard_map import shard_map
    from concourse import bass2jax as b2j

    b2j.install_neuronx_cc_hook()
    n_cores = len(in_maps)
    partition_name = nc.partition_id_tensor.name if nc.partition_id_tensor else None
    in_names, out_names, out_avals, zero_outs = [], [], [], []
    for alloc in nc.m.functions[0].allocations:
        if not isinstance(alloc, mybir.MemoryLocationSet):
            continue
        name = alloc.memorylocations[0].name
        if alloc.kind == "ExternalInput":
            if name != partition_name:
                in_names.append(name)
        elif alloc.kind == "ExternalOutput":
            out_names.append(name)
            shape = tuple(alloc.tensor_shape)
            dtype = mybir.dt.np(alloc.dtype)
            out_avals.append(jax.core.ShapedArray(shape, dtype))
            zero_outs.append(_np.zeros(shape, dtype))
    n_params = len(in_names)
    n_outs = len(out_avals)
    all_in_names = list(in_names) + out_names
    if partition_name is not None:
        all_in_names.append(partition_name)

    def _body(*args):
        operands = list(args)
        if partition_name is not None:
            operands.append(b2j.partition_id_tensor())
        outs = b2j._bass_exec_p.bind(
            *operands,
            out_avals=tuple(out_avals),
            in_names=tuple(all_in_names),
            out_names=tuple(out_names),
            lowering_input_output_aliases=(),
            sim_require_finite=True,
            sim_require_nnan=True,
            nc=nc,
        )
        return tuple(outs)

    devices = jax.devices()[:n_cores]
    mesh = Mesh(_np.asarray(devices), ("core",))
    in_specs = (PartitionSpec("core"),) * (n_params + n_outs)
    out_specs = (PartitionSpec("core"),) * len(out_names)
    donate = tuple(range(n_params, n_params + n_outs))
    sharded = jax.jit(
        shard_map(_body, mesh=mesh, in_specs=in_specs, out_specs=out_specs,
                  check_rep=False),
        donate_argnums=donate, keep_unused=True)
    concat_in = [
        _np.concatenate([_np.asarray(in_maps[c][nm]) for c in range(n_cores)], axis=0)
        for nm in in_names
    ]
    from jax.sharding import NamedSharding
    shard = NamedSharding(mesh, PartitionSpec("core"))
    in_dev = [jax.device_put(a, shard) for a in concat_in]
    zshapes = [((n_cores * z.shape[0],) + z.shape[1:], z.dtype) for z in zero_outs]

    def call():
        zs = [jax.device_put(_np.zeros(s, d), shard) for s, d in zshapes]
        outs = sharded(*in_dev, *zs)
        jax.block_until_ready(outs)
        return outs

    return call


def time_kernel_reps(inputs, iters=8, reps=4):
    """Differential in-program repetition timing: builds reps=1 and reps=N
    modules, times both through the same dispatch path, and attributes
    (tN - t1)/(N-1) to one kernel execution."""
    import time
    in_maps = host_prep(inputs)
    best = {}
    for r in (1, reps):
        nc, _ = _get_module((), r)
        call = make_runner(nc, in_maps)
        call()
        ts = []
        for _ in range(iters):
            t0 = time.perf_counter()
            call()
            ts.append(time.perf_counter() - t0)
        ts.sort()
        best[r] = ts[:3]
    import numpy as _np
    t1 = _np.mean(best[1])
    tN = _np.mean(best[reps])
    return (tN - t1) / (reps - 1) * 1e9, t1 * 1e9, tN * 1e9


def time_kernel(inputs, iters=10):
    """Returns (best_ns, floor_ns): wall time of one kernel execution and of a
    null kernel through the same dispatch path."""
    import time
    nc, _ = _get_module(())
    in_maps = host_prep(inputs)
    call = make_runner(nc, in_maps)
    call()
    ts = []
    for _ in range(iters):
        t0 = time.perf_counter()
        call()
        ts.append(time.perf_counter() - t0)
    best = min(ts)

    # null kernel floor
    key = "_null"
    if key not in _CACHE:
        ncn = bacc.Bacc("TRN2", target_bir_lowering=False, debug=False, num_devices=NCORES)
        xi = ncn.dram_tensor("x", [128, 4], f32, kind="ExternalInput")
        yo = ncn.dram_tensor("y", [128, 4], f32, kind="ExternalOutput")
        with tile.TileContext(ncn) as tcn:
            with tcn.tile_pool(name="p", bufs=1) as pool:
                t = pool.tile([128, 4], f32)
                ncn.sync.dma_start(out=t[:], in_=xi[:])
                ncn.sync.dma_start(out=yo[:], in_=t[:])
        ncn.compile()
        _CACHE[key] = ncn
    ncn = _CACHE[key]
    calln = make_runner(ncn, [{"x": np.zeros((128, 4), np.float32)}] * NCORES)
    calln()
    tn = []
    for _ in range(iters):
        t0 = time.perf_counter()
        calln()
        tn.append(time.perf_counter() - t0)
    floor = min(tn)
    return best * 1e9, floor * 1e9

